# revision 2
# baseline (speedup 1.0000x reference)
"""Trainium2 Bass kernel for nn_JointConditionalDistributionBlock.

Math (see analysis):
  output = softmax(marginals(m_k), axis=1), where
  m_k[h1,h2,h3] = sum_{f1..f4} softmax_{f4}(j_k + B)[h,f] * P_X[f]
The KDE scalar j_k is constant over the whole tensor, and softmax is
shift-invariant, so it drops out exactly:  softmax(j_k + B) == softmax(B).
P_X = softmax_{f4}(outer(x + tpx_bias) + bias_X) is a tiny [12^4] table.

Device work = stream B = bias_Y_given_X ([12]^7 f32, ~143 MB) and compute,
per 12-wide row r=(h,f1,f2,f3):
    num(r) = sum_f4 exp(B[r,f4]) * px[f123,f4]
    den(r) = sum_f4 exp(B[r,f4])
    m(h)   = sum_{f123} num/den
Sharding: 1728 h-triples / 8 cores = 216 triples (17.9 MB) per core.

Layout trick: the host pre-transposes each shard so the softmax axis f4
sits on SBUF partitions (partition = t_local*12 + f4, free = (f1,f2,f3)).
The grouped sums over f4 then run on the TensorEngine as matmuls with a
block-diagonal ones stationary: W_s[(t,f4), 10s+t] = 1.  Twelve tiles
accumulate into one [120,1728] PSUM pair (den banks 0-3, num banks 4-7)
via start=False, so the softmax-normalize stage runs on full 120-partition
tiles: reciprocal_approx_fast + one fused tensor_tensor_reduce.
"""

import numpy as np

H_P, F_P, K = 3, 4, 12
D = H_P + F_P
N_CORES = 8
NTRIP = K ** H_P            # 1728 h-triples total
TPC = NTRIP // N_CORES      # 216 triples per core
FREE = K ** 3               # 1728 = (f1,f2,f3)
TPT = 10                    # triples per full tile -> 120 partitions
ROWS_FULL = TPT * K         # 120
# superblocks: slot lists of triples-per-tile
SBS = [[TPT] * 12, [TPT] * 9 + [6]]     # 120 + 96 = 216
CHUNKS = [(0, 512), (512, 512), (1024, 512), (1536, 192)]

_CACHE = {}


def _softmax_last(x):
    x = np.asarray(x, np.float32)
    m = x.max(axis=-1, keepdims=True)
    e = np.exp(x - m, dtype=np.float32)
    return e / e.sum(axis=-1, keepdims=True)


def _build_program():
    import concourse.bacc as bacc
    from concourse import mybir
    from concourse.tile import TileContext

    nc = bacc.Bacc("TRN2", target_bir_lowering=False, debug=False)
    f32 = mybir.dt.float32
    bf16 = mybir.dt.bfloat16

    xin = nc.dram_tensor("xin", [TPC * K, FREE], f32, kind="ExternalInput").ap()
    pxr = nc.dram_tensor("pxr", [ROWS_FULL, FREE], bf16, kind="ExternalInput").ap()
    wst = nc.dram_tensor("wst", [ROWS_FULL, 13, ROWS_FULL], bf16,
                         kind="ExternalInput").ap()
    mout = nc.dram_tensor("mout", [TPC, 1], f32, kind="ExternalOutput").ap()

    with TileContext(nc) as tc:
        with (
            tc.tile_pool(name="singles", bufs=1) as singles,
            tc.tile_pool(name="xp", bufs=4) as xp,
            tc.tile_pool(name="ep", bufs=3) as epool,
            tc.tile_pool(name="epp", bufs=3) as eppool,
            tc.tile_pool(name="qp", bufs=2) as qp,
            tc.tile_pool(name="ps", bufs=1, space="PSUM") as ps,
        ):
            px_s = singles.tile([ROWS_FULL, FREE], bf16)
            nc.sync.dma_start(out=px_s, in_=pxr)
            w_s = singles.tile([ROWS_FULL, 13, ROWS_FULL], bf16)
            nc.sync.dma_start(out=w_s, in_=wst)

            row = 0
            trip = 0
            for slots in SBS:
                ntrip_sb = sum(slots)
                den_p = ps.tile([ROWS_FULL, FREE], mybir.dt.float32)
                num_p = ps.tile([ROWS_FULL, FREE], mybir.dt.float32)
                last = len(slots) - 1
                for s, tpt in enumerate(slots):
                    P = tpt * K
                    x_t = xp.tile([ROWS_FULL, FREE], f32)
                    nc.sync.dma_start(out=x_t[:P], in_=xin[row:row + P, :])
                    e_t = epool.tile([ROWS_FULL, FREE], bf16)
                    nc.scalar.activation(
                        out=e_t[:P], in_=x_t[:P],
                        func=mybir.ActivationFunctionType.Exp)
                    ep_t = eppool.tile([ROWS_FULL, FREE], bf16)
                    nc.vector.tensor_mul(ep_t[:P], e_t[:P], px_s[:P])
                    w_idx = 12 if tpt == 6 else s
                    lhsT = w_s[:P, w_idx, :]
                    for c0, cn in CHUNKS:
                        nc.tensor.matmul(
                            den_p[:, c0:c0 + cn], lhsT, e_t[:P, c0:c0 + cn],
                            start=(s == 0), stop=(s == last))
                        nc.tensor.matmul(
                            num_p[:, c0:c0 + cn], lhsT, ep_t[:P, c0:c0 + cn],
                            start=(s == 0), stop=(s == last))
                    row += P
                recip_t = qp.tile([ROWS_FULL, FREE], mybir.dt.float32)
                nc.vector.reciprocal_approx_fast(
                    out=recip_t[:ntrip_sb], in_=den_p[:ntrip_sb])
                qv_t = qp.tile([ROWS_FULL, FREE], mybir.dt.float32)
                m_t = qp.tile([ROWS_FULL, 1], mybir.dt.float32)
                nc.vector.tensor_mul(qv_t[:ntrip_sb], num_p[:ntrip_sb],
                                     recip_t[:ntrip_sb])
                nc.vector.tensor_reduce(
                    out=m_t[:ntrip_sb], in_=qv_t[:ntrip_sb],
                    axis=mybir.AxisListType.X, op=mybir.AluOpType.add)
                nc.sync.dma_start(out=mout[trip:trip + ntrip_sb, :],
                                  in_=m_t[:ntrip_sb])
                trip += ntrip_sb

    nc.compile()
    return nc


def _host_tables(x, tpx_bias, bias_X):
    import ml_dtypes

    t = (np.asarray(x, np.float32) + np.asarray(tpx_bias, np.float32)[0])
    r = t[0]
    for n in range(1, F_P):
        r = r[..., None] * t[n]                      # [12,12,12,12]
    px = _softmax_last(r + np.asarray(bias_X, np.float32))
    pxT = np.ascontiguousarray(px.transpose(3, 0, 1, 2)).reshape(K, FREE)
    pxr = np.ascontiguousarray(np.tile(pxT, (TPT, 1))).astype(ml_dtypes.bfloat16)

    W = np.zeros((13, ROWS_FULL, ROWS_FULL), np.float32)
    for s in range(12):
        for t_ in range(TPT):
            W[s, t_ * K:(t_ + 1) * K, 10 * s + t_] = 1.0
    for t_ in range(6):
        W[12, t_ * K:(t_ + 1) * K, 90 + t_] = 1.0
    wst = np.ascontiguousarray(W.transpose(1, 0, 2)).astype(ml_dtypes.bfloat16)
    return pxr, wst


def kernel(x, context_x, context_y, H_bandwidth, tpx_bias, bias_Y_given_X,
           bias_X):
    from concourse.bass_utils import run_bass_kernel_spmd

    if "nc" not in _CACHE:
        _CACHE["nc"] = _build_program()
    nc = _CACHE["nc"]

    pxr, wst = _host_tables(x, tpx_bias, bias_X)

    B7 = np.ascontiguousarray(np.asarray(bias_Y_given_X, np.float32)).reshape(
        NTRIP, K, K, K, K)
    in_maps = []
    for c in range(N_CORES):
        shard = B7[c * TPC:(c + 1) * TPC]            # [216, f1,f2,f3,f4]
        xc = np.ascontiguousarray(shard.transpose(0, 4, 1, 2, 3)).reshape(
            TPC * K, FREE)                           # row = t*12+f4
        in_maps.append({"xin": xc, "pxr": pxr, "wst": wst})

    res = run_bass_kernel_spmd(nc, in_maps, list(range(N_CORES)))
    m_flat = np.concatenate(
        [np.asarray(res.results[c]["mout"], np.float32)[:, 0]
         for c in range(N_CORES)])
    m_k = m_flat.reshape(K, K, K)

    marginals = np.stack([
        m_k.sum(axis=(1, 2)), m_k.sum(axis=(0, 2)), m_k.sum(axis=(0, 1))
    ]).astype(np.float32)
    return _softmax_last(marginals).astype(np.float32)
